# revision 28
# baseline (speedup 1.0000x reference)
"""Trainium2 Bass kernel for nn_AddNoise (segment_reduce category).

Math notes
----------
The reference materializes n = relu(conv2(bilinear(relu(conv1(broadcast(
text_noise)))))) for all B*171 classes and contracts it with a one-hot mask.
This kernel exploits:

 * conv1's input is spatially constant per (b, t): a 3x3 SAME conv of a
   constant 12x12 image has only 9 distinct outputs (interior/edge/corner
   regions) -> conv1 collapses to 9 partial-tap-sum matrices.
 * bilinear 12->24 (align_corners=False) of a piecewise-constant image is a
   fixed 9-coefficient linear map per output pixel (separable; the exact
   resize row-matrix is reproduced in closed form).
 * the einsum with the one-hot mask is a per-pixel class gather, and the
   target map holds only a handful of distinct classes per batch ("slots"),
   so only those slots are ever computed.
 * conv2 becomes a gathered matmul: a host-built sparse-as-dense matrix S
   maps (slot, region) vectors to the 9-tap stacks of every output pixel.
 * the data-dependent nan/deficit logic zeroes whole batches; it is host
   integer bookkeeping, and the compiled graph drops all work for zeroed
   batches.

Distribution (8 cores, SPMD - identical graph, per-core data):
 * conv1 region matmuls sharded by region k -> AllGather of the tiny V.
 * gathered conv2 sharded by (cout tile x spatial half) -> AllGather of the
   noise image (for the guidance convs) + an AllToAll that routes each
   producer's quarter-columns to the image-normalization cores, with
   data-driven masks making the routing SPMD-uniform.
 * guidance convs sharded by output channel; image/text normalizations by
   (batch x quarter) / (batch x channel tile).  Weights travel as bf16 (fp32
   PSUM accumulation); activations and outputs stay fp32.
"""

import os
import sys

import numpy as np

for _p in ('/opt/trn_rl_repo', '/root/.axon_site/_ro/trn_rl_repo'):
    if os.path.isdir(_p) and _p not in sys.path:
        sys.path.insert(0, _p)

import ml_dtypes

BF16 = ml_dtypes.bfloat16

B, T, C, H, W = 2, 171, 512, 24, 24
H2 = 12
STRIDE = 16
NCORES = 8
HW = H * W
TAPS = [(dh, dw) for dh in (-1, 0, 1) for dw in (-1, 0, 1)]


# ======================================================================
# Host-side preprocessing
# ======================================================================

def _host_prep(inputs):
    targets = np.asarray(inputs['targets'])
    tgt_small = targets[:, ::STRIDE, ::STRIDE]

    count_down = np.zeros(B, np.int64)
    for b in range(B):
        vals = tgt_small[b][(tgt_small[b] >= 0) & (tgt_small[b] < T)]
        count_down[b] = len(np.unique(vals))
    count_full = np.zeros(B, np.int64)
    for b in range(B):
        counts = np.bincount(
            np.clip(targets[b].reshape(-1), 0, 255), minlength=256)[:255]
        count_full[b] = int(np.sum(counts > 0))
    nan = (count_down != count_full) | (count_full == 0)
    deficit = max(B // 2 - int(np.sum(nan)), 0)
    rank = np.cumsum(~nan)
    extra = (~nan) & (rank <= deficit)
    zero_b = nan | extra
    active = [b for b in range(B) if not zero_b[b]]

    slots = []
    for b in range(B):
        vals = np.unique(tgt_small[b])
        vals = vals[(vals >= 0) & (vals < T)]
        slots.append(vals)
    NSLOT = max([len(slots[b]) for b in active], default=1)
    NSLOT = max(NSLOT, 1)
    assert NSLOT <= 128, "slot count above 128 not implemented"
    slot_map = np.full((B, H, W), -1, np.int64)
    for b in range(B):
        lut = {int(c): i for i, c in enumerate(slots[b])}
        for i in range(H):
            for j in range(W):
                slot_map[b, i, j] = lut.get(int(tgt_small[b, i, j]), -1)

    w1 = np.asarray(inputs['conv1_w'], np.float32)
    rowsets = [(1, 2), (0, 1, 2), (0, 1)]
    W1r = np.zeros((9, C, C), np.float32)
    for kr in range(3):
        for kc in range(3):
            m = np.zeros((C, C), np.float32)
            for r in rowsets[kr]:
                for c_ in rowsets[kc]:
                    m += w1[:, :, r, c_]
            W1r[kr * 3 + kc] = m.T                       # (cin, cout)

    # bilinear 12->24 row matrix (verified identical to jax.image.resize)
    Rm = np.zeros((H, H2), np.float32)
    for i in range(H):
        src = (i + 0.5) / 2.0 - 0.5
        p0 = int(np.floor(src))
        fr = src - p0
        wts = {p0: 1 - fr, p0 + 1: fr}
        tot = sum(v for p, v in wts.items() if 0 <= p < H2)
        for p, v in wts.items():
            if 0 <= p < H2:
                Rm[i, p] = v / tot
    RC = np.zeros((H, 3), np.float32)
    for p in range(H2):
        cls = 0 if p == 0 else (2 if p == H2 - 1 else 1)
        RC[:, cls] += Rm[:, p]

    w2 = np.asarray(inputs['conv2_w'], np.float32)
    W2r = np.transpose(w2, (2, 3, 1, 0)).reshape(9 * C, C).copy()

    gw = np.concatenate([np.asarray(inputs['g0_w'], np.float32),
                         np.asarray(inputs['g1_w'], np.float32),
                         np.asarray(inputs['g2_w'], np.float32)], axis=0)
    GW = np.transpose(gw, (2, 3, 1, 0)).reshape(9, C, 896).copy()
    gb = np.concatenate([np.asarray(inputs['g0_b'], np.float32),
                         np.asarray(inputs['g1_b'], np.float32),
                         np.asarray(inputs['g2_b'], np.float32)])

    tn = np.asarray(inputs['text_noise'], np.float32)
    Xs = {}
    for b in active:
        X = np.zeros((C, NSLOT), np.float32)
        for si, cls in enumerate(slots[b]):
            X[:, si] = tn[b, cls, 0, :]
        Xs[b] = X

    NB = len(active)
    # gathered-V row index (per batch) for (region k, slot s):
    #   core c = k % 8 computes k at j = k // 8.
    #   row = c*(2*NSLOT) + j*NSLOT + s
    KDIM = NCORES * 2 * NSLOT if NB else 0
    Scat = {}
    for bi, b in enumerate(active):
        S_b = np.zeros((9, KDIM, HW), np.float32)
        sm = slot_map[b]
        for ti, (dh, dw) in enumerate(TAPS):
            for i in range(H):
                ii = i + dh
                if not (0 <= ii < H):
                    continue
                for j in range(W):
                    jj = j + dw
                    if not (0 <= jj < W):
                        continue
                    s = sm[i, j]
                    if s < 0:
                        continue
                    for kr in range(3):
                        rc = RC[ii, kr]
                        if rc == 0.0:
                            continue
                        for kc in range(3):
                            cc = RC[jj, kc]
                            if cc == 0.0:
                                continue
                            k = kr * 3 + kc
                            row = (k % 8) * 2 * NSLOT \
                                + (k // 8) * NSLOT + s
                            S_b[ti, row, i * W + j] += rc * cc
        Scat[bi] = S_b

    return dict(zero_b=zero_b, active=active, slots=slots, NSLOT=NSLOT,
                KDIM=KDIM, slot_map=slot_map, W1r=W1r, Scat=Scat, W2r=W2r,
                GW=GW, gb=gb, Xs=Xs,
                b1=np.asarray(inputs['conv1_b'], np.float32),
                b2=np.asarray(inputs['conv2_b'], np.float32))


# ======================================================================
# Blob layouts (identical across cores; contents differ per core)
# ======================================================================

class Layout:
    def __init__(self):
        self.col = 0
        self.regions = {}

    def add(self, name, rows, cols):
        assert name not in self.regions, name
        self.regions[name] = (self.col, rows, cols)
        self.col += cols

    def sl(self, name):
        return self.regions[name]


def _make_meta(prep):
    active = prep['active']
    NB = len(active)
    NSLOT = prep['NSLOT']
    KDIM = prep['KDIM']
    KT = (KDIM + 127) // 128 if NB else 0

    assert NB <= 1, "deficit logic guarantees at most one active batch"
    meta = dict(NB=NB, active=active, NSLOT=NSLOT, KDIM=KDIM, KT=KT)
    if NB == 1:
        meta['nz_cols'] = 288
        meta['g_subs'] = [112]
    elif NB == 2:
        meta['nz_cols'] = 576
        meta['g_subs'] = [128, 96]
    else:
        meta['nz_cols'] = 0
        meta['g_subs'] = []
    meta['s_cols'] = 9 * meta['nz_cols']
    meta['n_inact'] = B - NB
    meta['b1_nz'] = bool(np.any(prep['b1']))

    lw0 = Layout()                                  # bf16 blob, needed first
    for bi in range(NB):
        for j in range(2):
            for kt in range(4):
                lw0.add(f'w1r_{bi}_{j}_{kt}', 128, 512)
    for bi in range(NB):
        for kt in range(4):
            lw0.add(f'xt_{bi}_{kt}', 128, NSLOT)
    if lw0.col == 0:
        lw0.add('pad_bw0', 1, 4)
    meta['lw0'] = lw0
    lw = Layout()                                   # bf16 blob, bulk
    for ti in range(9):
        for mt in range(KT):
            lw.add(f's2_{ti}_{mt}', 128, meta['nz_cols'])
    if NB:
        for ti in range(9):
            for m in range(4):
                lw.add(f'w2t_{ti}_{m}', 128, 128)
        for ti in range(9):
            for m in range(4):
                for si, sw in enumerate(meta['g_subs']):
                    lw.add(f'gw_{ti}_{m}_{si}', 128, sw)
    if lw.col == 0:
        lw.add('pad_bw', 1, 4)
    meta['lw'] = lw

    lf = Layout()                                   # f32 blob
    if meta['b1_nz']:
        lf.add('b1row', 1, 512)
    lf.add('bias', 128, 4)       # col0 b2-slice, col1/2 gb subtile slices
    if NB:
        lf.add('a2amask', 128, NCORES)
    for m in range(4):
        lf.add(f'img_{m}', 128, 144)
    lf.add('tf', 128, T)
    lf.add('tn', 128, T)
    for si, sw in enumerate(meta['g_subs']):
        lf.add(f'visa_{si}', 128, 576)
    for ib in range(meta['n_inact']):
        lf.add(f'visi_{ib}', 128, 576)
    meta['lf'] = lf

    lo = Layout()                                   # f32 out blob
    lo.add('txt_out', 128, T)
    for ib in range(meta['n_inact']):
        lo.add(f'v_in_{ib}', 128, 576)
    meta['early_cols'] = lo.col
    for m in range(4):
        lo.add(f'img_out_{m}', 128, 144)
    for si, sw in enumerate(meta['g_subs']):
        lo.add(f'v_act_{si}', 128, 576)
    meta['lo'] = lo
    return meta


def _pack_core(core, inputs, prep, meta):
    NB, active = meta['NB'], meta['active']
    NSLOT, KT, KDIM = meta['NSLOT'], meta['KT'], meta['KDIM']
    lw, lf = meta['lw'], meta['lf']
    nz_cols, s_cols = meta['nz_cols'], meta['s_cols']

    lw0 = meta['lw0']
    bw0 = np.zeros((128, max(lw0.col, 1)), BF16)
    bw = np.zeros((128, max(lw.col, 1)), BF16)
    bf = np.zeros((128, max(lf.col, 1)), np.float32)

    def put_w(name, arr):
        if name in lw0.regions:
            c0, rows, cols = lw0.sl(name)
            bw0[:arr.shape[0], c0:c0 + arr.shape[1]] = arr.astype(BF16)
            return
        c0, rows, cols = lw.sl(name)
        bw[:arr.shape[0], c0:c0 + arr.shape[1]] = arr.astype(BF16)

    def put_f(name, arr):
        c0, rows, cols = lf.sl(name)
        bf[:arr.shape[0], c0:c0 + arr.shape[1]] = arr

    for bi in range(NB):
        for j in range(2):
            k = core + 8 * j
            if k < 9:
                w = prep['W1r'][k]
                for kt in range(4):
                    put_w(f'w1r_{bi}_{j}_{kt}', w[kt * 128:(kt + 1) * 128])

    if NB:
        # noise unit: NB==1 -> (cout tile jt=core%4, half hh=core//4)
        #             NB==2 -> (batch-pos bu=core//4, cout tile jt=core%4)
        jt = core % 4
        if NB == 1:
            bu, hh = 0, core // 4
        else:
            bu, hh = core // 4, None
        S = prep['Scat'][bu]
        for ti in range(9):
            for mt in range(KT):
                r0, r1 = mt * 128, min((mt + 1) * 128, KDIM)
                sh = S[ti][r0:r1]
                if NB == 1:
                    sh = sh.reshape(r1 - r0, H, W)[
                        :, :, hh * 12:(hh + 1) * 12].reshape(r1 - r0, 288)
                put_w(f's2_{ti}_{mt}', sh)
        for ti in range(9):
            for m in range(4):
                blk = prep['W2r'][ti * C + m * 128: ti * C + (m + 1) * 128,
                                  jt * 128:(jt + 1) * 128]
                put_w(f'w2t_{ti}_{m}', blk)
        if NB == 1:
            co0 = core * 112
        else:
            co0 = (core % 4) * 224
        off = 0
        for si, sw in enumerate(meta['g_subs']):
            for ti in range(9):
                for m in range(4):
                    blk = prep['GW'][ti][m * 128:(m + 1) * 128,
                                         co0 + off:co0 + off + sw]
                    put_w(f'gw_{ti}_{m}_{si}', blk)
            off += sw

    for bi in range(NB):
        X = prep['Xs'][active[bi]]
        for kt in range(4):
            put_w(f'xt_{bi}_{kt}', X[kt * 128:(kt + 1) * 128])
    # ---- f32 blob ----
    if meta['b1_nz']:
        put_f('b1row', prep['b1'][None, :])

    bias = np.zeros((128, 4), np.float32)
    if NB:
        jt = core % 4
        bias[:, 0] = prep['b2'][jt * 128:(jt + 1) * 128]
        co0 = core * 112 if NB == 1 else (core % 4) * 224
        off = 0
        for si, sw in enumerate(meta['g_subs']):
            bias[:sw, 1 + si] = prep['gb'][co0 + off:co0 + off + sw]
            off += sw
    put_f('bias', bias)

    if NB:
        # AllToAll sender masks: my piece goes to receiver c' iff my half
        # (NB==1) / my batch (NB==2) matches the receiver's image unit.
        mask = np.zeros((128, NCORES), np.float32)
        for cd in range(NCORES):
            b_r, q_r = cd // 4, cd % 4
            if NB == 1:
                ok = (b_r == active[0])
            else:
                ok = (b_r == active[core // 4])
            mask[:, cd] = 1.0 if ok else 0.0
        put_f('a2amask', mask)

    b_img, q = core // 4, core % 4
    imgf = np.asarray(inputs['image_features'], np.float32).reshape(B, C, HW)
    for m in range(4):
        put_f(f'img_{m}', imgf[b_img, m * 128:(m + 1) * 128,
                               q * 144:(q + 1) * 144])

    b_txt, ct = core // 4, core % 4
    tf_ = np.asarray(inputs['text_features'], np.float32)[b_txt, :, 0, :].T
    tn_ = np.asarray(inputs['text_noise'], np.float32)[b_txt, :, 0, :].T
    put_f('tf', tf_[ct * 128:(ct + 1) * 128])
    put_f('tn', tn_[ct * 128:(ct + 1) * 128])

    vis_cat = np.concatenate(
        [np.asarray(inputs['vis0'], np.float32).reshape(B, 512, HW),
         np.asarray(inputs['vis1'], np.float32).reshape(B, 256, HW),
         np.asarray(inputs['vis2'], np.float32).reshape(B, 128, HW)], axis=1)
    if NB:
        ba = active[0] if NB == 1 else active[core // 4]
        co0 = core * 112 if NB == 1 else (core % 4) * 224
        off = 0
        for si, sw in enumerate(meta['g_subs']):
            put_f(f'visa_{si}', vis_cat[ba, co0 + off:co0 + off + sw])
            off += sw
    inact = [b for b in range(B) if b not in active]
    for ib, b in enumerate(inact):
        co0 = core * 112
        put_f(f'visi_{ib}', vis_cat[b, co0:co0 + 112])

    return {'bw0': bw0, 'bw': bw, 'bf': bf}


# ======================================================================
# Graph builder (SPMD: one graph for all 8 cores)
# ======================================================================

def _build_graph(meta):
    import concourse.mybir as mybir
    from concourse import bacc, tile

    from concourse.tile_rust import add_dep_helper
    f32 = mybir.dt.float32
    bf16 = mybir.dt.bfloat16
    Relu = mybir.ActivationFunctionType.Relu
    Square = mybir.ActivationFunctionType.Square
    Copy = mybir.ActivationFunctionType.Copy

    NB, NSLOT, KT, KDIM = meta['NB'], meta['NSLOT'], meta['KT'], meta['KDIM']
    lw0, lw, lf, lo = meta['lw0'], meta['lw'], meta['lf'], meta['lo']
    nz_cols, s_cols = meta['nz_cols'], meta['s_cols']
    g_subs = meta['g_subs']

    nc = bacc.Bacc("TRN2", target_bir_lowering=False, debug=False,
                   num_devices=NCORES)
    bw0_p = nc.dram_tensor("bw0", [128, max(lw0.col, 1)], bf16,
                           kind="ExternalInput")
    bw_p = nc.dram_tensor("bw", [128, max(lw.col, 1)], bf16,
                          kind="ExternalInput")
    bf_p = nc.dram_tensor("bf", [128, max(lf.col, 1)], f32,
                          kind="ExternalInput")
    out_p = nc.dram_tensor("out", [128, max(lo.col, 1)], f32,
                           kind="ExternalOutput")

    with tile.TileContext(nc) as tc:
        with (
            tc.tile_pool(name="pin", bufs=1) as pin,
            tc.tile_pool(name="prot", bufs=3) as prot,
            tc.tile_pool(name="pdram", bufs=1, space="DRAM") as pdram,
            tc.tile_pool(name="ppsum", bufs=2, space="PSUM") as ppsum,
        ):
            bw0_t = pin.tile([128, max(lw0.col, 1)], bf16, name="bw0_t")
            bw_t = pin.tile([128, max(lw.col, 1)], bf16, name="bw_t")
            bf_t = pin.tile([128, max(lf.col, 1)], f32, name="bf_t")
            out_t = pin.tile([128, max(lo.col, 1)], f32, name="out_t")
            if NB:
                c_j0, _, w_j0 = lw0.sl('w1r_0_0_0')
                j0_end = lw0.sl('w1r_0_0_3')[0] + 512
                nc.sync.dma_start(out=bw0_t[:, :j0_end],
                                  in_=bw0_p[:, :j0_end])
                nc.sync.dma_start(out=bw0_t[:, j0_end:],
                                  in_=bw0_p[:, j0_end:])
            else:
                nc.sync.dma_start(out=bw0_t[:], in_=bw0_p[:])
            nc.scalar.dma_start(out=bf_t[:], in_=bf_p[:])
            nc.sync.dma_start(out=bw_t[:], in_=bw_p[:])

            def wsl(name, rows=128):
                if name in lw0.regions:
                    c0, r, cols = lw0.sl(name)
                    return bw0_t[:rows, c0:c0 + cols]
                c0, r, cols = lw.sl(name)
                return bw_t[:rows, c0:c0 + cols]

            def fsl(name, rows=128):
                c0, r, cols = lf.sl(name)
                return bf_t[:rows, c0:c0 + cols]

            def osl(name, rows=128):
                c0, r, cols = lo.sl(name)
                return out_t[:rows, c0:c0 + cols]

            nc.gpsimd.memset(out_t[:], 0.0)
            ones_c = pin.tile([128, 1], f32, name="ones_c")
            nc.vector.memset(ones_c[:], 1.0)
            ones_r = pin.tile([1, 128], f32, name="ones_r")
            nc.vector.memset(ones_r[:], 1.0)
            if NB and meta['b1_nz']:
                ones_s = pin.tile([1, NSLOT], f32, name="ones_s")
                nc.vector.memset(ones_s[:], 1.0)

            # PE warmup filler (keeps the HAM clock-gate open)
            warm_src = pin.tile([128, 512], bf16, name="warm_src")
            nc.vector.memset(warm_src[:], 1.0)

            warm_sink = pin.tile([1, 64], f32, name="warm_sink")

            def warm(n, dep=None):
                wp = None
                for _ in range(n):
                    wp = ppsum.tile([128, 512], f32, name="warm_ps",
                                    tag="ug_ps")
                    lhs = dep if dep is not None else warm_src[:, 0:128]
                    kk = lhs.shape[0]
                    nc.tensor.matmul(wp[:], lhs, warm_src[:kk, :],
                                     start=True, stop=True)
                if wp is not None:
                    nc.scalar.activation(warm_sink[:, 0:4], wp[0:1, 0:4],
                                         Copy)

            warm(8)

            if NB:
                agv_in = pdram.tile([NB, 4, 128, 2, NSLOT], bf16,
                                    name="agv_in")
                agv_out = pdram.tile([NCORES, NB, 4, 128, 2, NSLOT], bf16,
                                     name="agv_out", addr_space="Shared")
                agn_in = pdram.tile([128, nz_cols], bf16, name="agn_in")
                agn_out = pdram.tile([NCORES, 128, nz_cols], bf16,
                                     name="agn_out", addr_space="Shared")
                pc = 72 if NB == 1 else 144       # a2a piece columns
                a2a_in = pdram.tile([NCORES, 128, pc], bf16, name="a2a_in")
                a2a_out = pdram.tile([NCORES, 128, pc], bf16,
                                     name="a2a_out")

                # ---------- V: conv1 region vectors, channel-major ----------
                # vshT columns ordered (b, cout-tile ct, j, s); the AllGather
                # carries V^T so the Q stage can use it as lhsT directly.
                vshT = pin.tile([128, NB * 4 * 2 * NSLOT], bf16, name="vshT")
                for bi in range(NB):
                    for ct in range(4):
                        for j in range(2):
                            vp = ppsum.tile([128, NSLOT], f32, name="v_ps",
                                            tag="v_ps", bufs=2)
                            first = True
                            if meta['b1_nz']:
                                nc.tensor.matmul(
                                    vp[:], fsl('b1row', rows=1)[
                                        :, ct * 128:(ct + 1) * 128],
                                    ones_s[:], start=True, stop=False)
                                first = False
                            for kt in range(4):
                                v_mm_last = nc.tensor.matmul(
                                    vp[:],
                                    wsl(f'w1r_{bi}_{j}_{kt}')[
                                        :, ct * 128:(ct + 1) * 128],
                                    wsl(f'xt_{bi}_{kt}'),
                                    start=first, stop=(kt == 3))
                                first = False
                            blk = (bi * 4 + ct) * 2 + j
                            v_relu_last = nc.scalar.activation(
                                vshT[:, blk * NSLOT:(blk + 1) * NSLOT],
                                vp[:], Relu)
                nc.scalar.dma_start(
                    out=agv_in.rearrange("b kt p j s -> p b kt j s"),
                    in_=vshT.rearrange("p (b kt j s) -> p b kt j s",
                                       b=NB, kt=4, j=2))
                nc.gpsimd.collective_compute(
                    "AllGather", mybir.AluOpType.bypass,
                    replica_groups=[list(range(NCORES))],
                    ins=[agv_in[:]], outs=[agv_out[:]])

            # ---------- text unit (fills the AllGather gap) ----------
            tsq = pin.tile([128, T], f32, name="tsq")
            tss = pin.tile([128, 8], f32, name="tss")
            txn = pin.tile([128, T], f32, name="txn")

            def norm_free(src, dst, c0):
                sq_i = nc.scalar.activation(tsq[:], src, Square,
                                            accum_out=tss[:, c0:c0 + 1])
                if NB and norm_free.first:
                    norm_free.first = False
                    add_dep_helper(sq_i.ins, v_relu_last.ins, sync=False,
                                   reason="keep V->AllGather chain ahead "
                                          "of text norm on ACT")
                nc.scalar.sqrt(tss[:, c0 + 1:c0 + 2], tss[:, c0:c0 + 1])
                nc.vector.tensor_scalar_max(tss[:, c0 + 1:c0 + 2],
                                            tss[:, c0 + 1:c0 + 2], 1e-12)
                nc.vector.reciprocal(tss[:, c0 + 2:c0 + 3],
                                     tss[:, c0 + 1:c0 + 2])
                nc.vector.tensor_scalar_mul(dst, src, tss[:, c0 + 2:c0 + 3])

            norm_free.first = True
            norm_free(fsl('tf'), txn[:], 0)
            nc.vector.tensor_add(txn[:], txn[:], fsl('tn'))
            norm_free(txn[:], osl('txt_out'), 4)

            # ---------- image norm1 ----------
            imn = [pin.tile([128, 144], f32, name=f"imn_{m}")
                   for m in range(4)]

            def img_norm(srcs, dsts, tagsfx):
                ssp = ppsum.tile([1, 144], f32, name="ss_ps",
                                 tag="ssbc_ps", bufs=1)
                for m in range(4):
                    sq = prot.tile([128, 144], f32, name="isq", tag="isq")
                    sq_i = nc.scalar.square(sq[:], srcs[m])
                    mm_i = nc.tensor.matmul(ssp[:], ones_c[:], sq[:],
                                            start=(m == 0), stop=(m == 3))
                    if NB and img_norm.first:
                        img_norm.first = False
                        add_dep_helper(sq_i.ins, v_relu_last.ins, sync=False,
                                       reason="keep V chain ahead of img "
                                              "norm on ACT")
                        add_dep_helper(mm_i.ins, v_mm_last.ins, sync=False,
                                       reason="keep V matmuls ahead of img "
                                              "norm on PE")
                ssb = pin.tile([1, 144], f32, name=f"ssb{tagsfx}")
                nc.scalar.sqrt(ssb[:], ssp[:])
                nc.vector.tensor_scalar_max(ssb[:], ssb[:], 1e-12)
                nc.vector.reciprocal(ssb[:], ssb[:])
                bcp = ppsum.tile([128, 144], f32, name="bc_ps",
                                 tag="ssbc_ps", bufs=1)
                nc.tensor.matmul(bcp[:], ones_r[:], ssb[:],
                                 start=True, stop=True)
                for m in range(4):
                    nc.vector.tensor_mul(dsts[m], srcs[m], bcp[:])

            img_norm.first = True
            img_srcs = [fsl(f'img_{m}') for m in range(4)]
            if NB:
                img_norm(img_srcs, [t[:] for t in imn], "1")
            else:
                img_norm(img_srcs,
                         [osl(f'img_out_{m}') for m in range(4)], "1")

            # ---------- noise pipeline (Q factorization) ----------
            if NB:
                # vbufT: V^T, cin-tile blocks along the free axis; block kt
                # holds rows cin [128kt,+128) x m=(core,j,s) of the single
                # active batch (NB is provably <= 1).  One DMA for all four.
                # kt-major flat layout (the matmul stationary operand must
                # have a single free dimension); per-kt DMAs on alternating
                # HWDGE rings so they pipeline.
                vbufT = pin.tile([128, 4 * KDIM], bf16, name="vbufT")
                for kt in range(4):
                    eng = nc.scalar if kt % 2 == 0 else nc.sync
                    eng.dma_start(
                        out=vbufT[:, kt * KDIM:(kt + 1) * KDIM]
                        .rearrange("p (c j s) -> p c j s",
                                   c=NCORES, j=2),
                        in_=agv_out.rearrange(
                            "c b kt p j s -> b kt p c j s")[0, kt])

                warm(8, dep=vshT[0:128, 0:128])

                # Q[(tap, m-tile)] = V^T-tile.T @ W2r-slice -> (m, cout) bf16
                qt = pin.tile([128, 9 * KT * 128], bf16, name="qt")
                for ti in range(9):
                    for mt in range(KT):
                        rows = min(128, KDIM - mt * 128)
                        qp = ppsum.tile([128, 128], f32, name="ug_ps",
                                        tag="ug_ps")
                        for kt in range(4):
                            c0 = kt * KDIM + mt * 128
                            nc.tensor.matmul(
                                qp[:rows, :], vbufT[:, c0:c0 + rows],
                                wsl(f'w2t_{ti}_{kt}'),
                                start=(kt == 0), stop=(kt == 3))
                        kk = ti * KT + mt
                        nc.any.tensor_copy(
                            qt[:rows, kk * 128:(kk + 1) * 128], qp[:rows, :])

                # final conv2: contract (tap, m) against the S tiles
                nzt = pin.tile([128, nz_cols], bf16, name="nzt")
                for h0 in range(0, nz_cols, 288):
                    hn = min(288, nz_cols - h0)
                    npp = ppsum.tile([128, 288], f32, name="nz_ps",
                                     tag="nz_ps", bufs=1)
                    first = True
                    for ti in range(9):
                        for mt in range(KT):
                            rows = min(128, KDIM - mt * 128)
                            kk = ti * KT + mt
                            nc.tensor.matmul(
                                npp[:, :hn],
                                qt[:rows, kk * 128:(kk + 1) * 128],
                                wsl(f's2_{ti}_{mt}',
                                    rows=rows)[:, h0:h0 + hn],
                                start=first,
                                stop=(ti == 8 and mt == KT - 1))
                            first = False
                    nc.scalar.activation(nzt[:, h0:h0 + hn], npp[:, :hn],
                                         Relu, bias=fsl('bias')[:, 0:1])
                nc.scalar.dma_start(out=agn_in[:], in_=nzt[:])
                nc.gpsimd.collective_compute(
                    "AllGather", mybir.AluOpType.bypass,
                    replica_groups=[list(range(NCORES))],
                    ins=[agn_in[:]], outs=[agn_out[:]])

                # AllToAll: route my piece of each image quarter to the
                # image units.  NB==1: my shard is a W-half (i-major x 12
                # cols), receiver quarter rows i in [6q,6q+6) -> cols
                # [72q, 72q+72).  NB==2: full hw -> cols [144q, 144q+144).
                a2a_sb = pin.tile([128, NCORES * pc], bf16, name="a2a_sb")
                for cd in range(NCORES):
                    coloff = pc * (cd % 4)
                    nc.scalar.activation(
                        a2a_sb[:, cd * pc:(cd + 1) * pc],
                        nzt[:, coloff:coloff + pc], Copy,
                        scale=fsl('a2amask')[:, cd:cd + 1])
                nc.scalar.dma_start(
                    out=a2a_in.rearrange("d p c -> p d c"),
                    in_=a2a_sb.rearrange("p (d c) -> p d c", d=NCORES))
                nc.gpsimd.collective_compute(
                    "AllToAll", mybir.AluOpType.bypass,
                    replica_groups=[list(range(NCORES))],
                    ins=[a2a_in[:]], outs=[a2a_out[:]])

                warm(8, dep=nzt[0:128, 0:128])

                # ---------- padded noise for the guidance convs ----------
                pad = [pin.tile([128, 26 * 26], bf16, name=f"pad_{m}")
                       for m in range(4)]
                for m in range(4):
                    nc.gpsimd.memset(pad[m][:], 0.0)
                if NB == 1:
                    # per-cin-tile loads (shards m and m+4), alternating
                    # rings so the first pad copies start ~1us earlier
                    nfull = pin.tile([128, 8 * 288], bf16, name="nfull")
                    nfv = nfull.rearrange("p (u c) -> p u c", u=8)
                    agv2 = agn_out.rearrange("u p c -> p u c")
                    for m in range(4):
                        eng = nc.scalar if m % 2 == 0 else nc.sync
                        eng.dma_start(out=nfv[:, m::4, :],
                                      in_=agv2[:, m::4, :])
                        for h in range(2):
                            u = h * 4 + m
                            dst = pad[m].rearrange(
                                "p (r c) -> p r c", r=26)[
                                :, 1:25, 1 + h * 12:13 + h * 12]
                            nc.any.tensor_copy(
                                dst,
                                nfull[:, u * 288:(u + 1) * 288].rearrange(
                                    "p (r c) -> p r c", r=24))
                else:
                    # NB==2: shards are (batch-pos bu, cout tile m) with full
                    # hw; my batch's shard is selected by data (mask sums).
                    for m in range(4):
                        t0 = prot.tile([128, 576], bf16, name="nt2a",
                                       tag="nt2a")
                        nc.sync.dma_start(out=t0[:], in_=agn_out[m])
                        t1 = prot.tile([128, 576], bf16, name="nt2b",
                                       tag="nt2b")
                        nc.sync.dma_start(out=t1[:], in_=agn_out[4 + m])
                        tf_ = prot.tile([128, 576], f32, name="nt2f",
                                        tag="nt2f")
                        # my-batch selector: reuse a2amask columns: receiver
                        # cores 0..3 are batch 0, 4..7 batch 1.  mask of
                        # "shard bu matches my batch" equals a2amask[:, bu*4]
                        nc.vector.tensor_scalar(
                            tf_[:], t0[:], fsl('a2amask')[:, 0:1], None,
                            mybir.AluOpType.mult)
                        t1f = prot.tile([128, 576], f32, name="nt2g",
                                        tag="nt2g")
                        nc.vector.tensor_scalar(
                            t1f[:], t1[:], fsl('a2amask')[:, 4:5], None,
                            mybir.AluOpType.mult)
                        nc.vector.tensor_add(tf_[:], tf_[:], t1f[:])
                        dst = pad[m].rearrange("p (r c) -> p r c", r=26)[
                            :, 1:25, 1:25]
                        nc.any.tensor_copy(
                            dst, tf_.rearrange("p (r c) -> p r c", r=24))

                # ---------- guidance convs ----------
                for si, sw in enumerate(g_subs):
                    for h in range(2):
                        gp = ppsum.tile([sw, 288], f32, name="g_ps",
                                        tag="g_ps")
                        first = True
                        for ti, (dh, dw) in enumerate(TAPS):
                            for m in range(4):
                                rhs = pad[m].rearrange(
                                    "p (r c) -> p r c", r=26)[
                                    :, 1 + dh:25 + dh,
                                    1 + dw + h * 12:13 + dw + h * 12]
                                nc.tensor.matmul(
                                    gp[:], wsl(f'gw_{ti}_{m}_{si}'),
                                    rhs, start=first,
                                    stop=(ti == 8 and m == 3))
                                first = False
                        gr = prot.tile([sw, 288], f32, name="gr", tag="gr")
                        nc.scalar.activation(
                            gr[:], gp[:], Relu,
                            bias=fsl('bias', rows=sw)[:, 1 + si:2 + si])
                        dst = osl(f'v_act_{si}', rows=sw).rearrange(
                            "p (r c) -> p r c", r=24)[
                            :, :, h * 12:(h + 1) * 12]
                        nc.vector.tensor_add(
                            dst,
                            fsl(f'visa_{si}', rows=sw).rearrange(
                                "p (r c) -> p r c", r=24)[
                                :, :, h * 12:(h + 1) * 12],
                            gr.rearrange("p (r c) -> p r c", r=24))

                # ---------- image: add routed noise quarter + norm2 ----------
                arec = pin.tile([128, 8 * pc], bf16, name="arec")
                nc.scalar.dma_start(
                    out=arec.rearrange("p (u c) -> p u c", u=8),
                    in_=a2a_out.rearrange("u p c -> p u c"))
                for m in range(4):
                    ra = arec[:, m * pc:(m + 1) * pc]
                    rb = arec[:, (m + 4) * pc:(m + 5) * pc]
                    nq = prot.tile([128, 144], f32, name="nqf", tag="nqf")
                    if NB == 1:
                        # ranks m / m+4 hold j-halves 0 / 1 of my 6 rows
                        nqv = nq.rearrange("p (r c) -> p r c", c=24)
                        nc.any.tensor_copy(
                            nqv[:, :, 0:12],
                            ra.rearrange("p (r c) -> p r c", c=12))
                        nc.any.tensor_copy(
                            nqv[:, :, 12:24],
                            rb.rearrange("p (r c) -> p r c", c=12))
                    else:
                        nc.vector.tensor_add(nq[:], ra[:], rb[:])
                    nc.vector.tensor_add(imn[m][:], imn[m][:], nq[:])
                img_norm([t[:] for t in imn],
                         [osl(f'img_out_{m}') for m in range(4)], "2")

            # ---------- vis passthrough for inactive batches ----------
            if meta['n_inact']:
                relugb = pin.tile([128, 1], f32, name="relugb")
                nc.scalar.activation(relugb[:112, :],
                                     fsl('bias', rows=112)[:, 1:2],
                                     Relu)
                for ib in range(meta['n_inact']):
                    nc.vector.tensor_scalar_add(
                        osl(f'v_in_{ib}', rows=112),
                        fsl(f'visi_{ib}', rows=112),
                        relugb[:112, :])

            ec = meta['early_cols']
            if 0 < ec < lo.col and NB:
                imgc = lo.sl('img_out_0')[0]
                vac = lo.sl('v_act_0')[0]
                nc.scalar.dma_start(out=out_p[:, :ec], in_=out_t[:, :ec])
                nc.sync.dma_start(out=out_p[:, vac:], in_=out_t[:, vac:])
                nc.scalar.dma_start(out=out_p[:, imgc:vac],
                                    in_=out_t[:, imgc:vac])
            elif 0 < ec < lo.col:
                nc.scalar.dma_start(out=out_p[:, :ec], in_=out_t[:, :ec])
                nc.sync.dma_start(out=out_p[:, ec:], in_=out_t[:, ec:])
            else:
                nc.sync.dma_start(out=out_p[:], in_=out_t[:])

    nc.compile()
    return nc


# ======================================================================
# Unpack
# ======================================================================

def _unpack(results, inputs, prep, meta):
    active = meta['active']
    NB = meta['NB']
    lo = meta['lo']

    def get(core, name, rows):
        c0, r, cols = lo.sl(name)
        return results[core]['out'][:rows, c0:c0 + cols]

    img_out = np.zeros((B, C, HW), np.float32)
    for core in range(NCORES):
        b_img, q = core // 4, core % 4
        for m in range(4):
            img_out[b_img, m * 128:(m + 1) * 128, q * 144:(q + 1) * 144] = \
                get(core, f'img_out_{m}', 128)
    img_out = img_out.reshape(B, C, H, W)

    txt_out = np.zeros((B, C, T), np.float32)
    for core in range(NCORES):
        b_txt, ct = core // 4, core % 4
        txt_out[b_txt, ct * 128:(ct + 1) * 128, :] = get(core, 'txt_out', 128)
    txt_out = np.transpose(txt_out, (0, 2, 1))[:, :, None, :]

    v_cat = np.zeros((B, 896, HW), np.float32)
    for core in range(NCORES):
        if NB == 1:
            ba = active[0]
            co0 = core * 112
            v_cat[ba, co0:co0 + 112] = get(core, 'v_act_0', 112)
        elif NB == 2:
            ba = active[core // 4]
            co0 = (core % 4) * 224
            v_cat[ba, co0:co0 + 128] = get(core, 'v_act_0', 128)
            v_cat[ba, co0 + 128:co0 + 224] = get(core, 'v_act_1', 96)
        inact = [b for b in range(B) if b not in active]
        for ib, b in enumerate(inact):
            co0 = core * 112
            v_cat[b, co0:co0 + 112] = get(core, f'v_in_{ib}', 112)

    v0 = v_cat[:, :512].reshape(B, 512, H, W).copy()
    v1 = v_cat[:, 512:768].reshape(B, 256, H, W).copy()
    v2 = v_cat[:, 768:].reshape(B, 128, H, W).copy()
    return (img_out, txt_out, v0, v1, v2, np.zeros((), np.float32))


# ======================================================================
# Entry point
# ======================================================================

def kernel(**inputs):
    prep = _host_prep(inputs)
    meta = _make_meta(prep)
    packs = [_pack_core(c, inputs, prep, meta) for c in range(NCORES)]
    nc = _build_graph(meta)
    kernel.last_nc = nc

    backend = os.environ.get('KERNEL_BACKEND', 'hw')
    if backend == 'sim':
        from concourse.bass_interp import MultiCoreSim
        sim = MultiCoreSim(nc, NCORES)
        for c in range(NCORES):
            sim.cores[c].tensor('bw0')[:] = packs[c]['bw0']
            sim.cores[c].tensor('bw')[:] = packs[c]['bw']
            sim.cores[c].tensor('bf')[:] = packs[c]['bf']
        sim.simulate(check_with_hw=False)
        results = [{'out': np.asarray(sim.cores[c].mem_tensor('out'))}
                   for c in range(NCORES)]
        kernel.last_sim_time_ns = getattr(sim, 'global_time', None)
    else:
        from concourse.bass_utils import run_bass_kernel_spmd
        in_maps = [{'bw0': packs[c]['bw0'], 'bw': packs[c]['bw'],
                    'bf': packs[c]['bf']} for c in range(NCORES)]
        res = run_bass_kernel_spmd(nc, in_maps,
                                   core_ids=list(range(NCORES)))
        results = res.results
        kernel.last_exec_time_ns = res.exec_time_ns

    return _unpack(results, inputs, prep, meta)
